# revision 1
# baseline (speedup 1.0000x reference)
"""Trainium2 Bass kernel v2 for nn_MultiHeadAttention_67379446939752.

Per-token multi-head attention, data-parallel over tokens across 8 cores.

v2 changes vs baseline:
  - All tensor storage in fp16 (validated: relmax ~3e-3 vs 2e-2 budget);
    PSUM accumulation and the exp() intermediate stay f32.  Halves all HBM
    traffic, enables FWL (fast weight load) so ldweights is hidden, and
    fp16 matmuls run at 1 cycle/row at ANY free size (no f32r N>=256
    constraint, so the score matmul needs no 2-block window trick).
  - Phase B processes 4 token-blocks (32 tokens) per instruction group:
    one exp over [128,512], one grouped reduce for softmax sums, batched
    copies.  ~4x fewer DVE/Act instructions and coarser cross-engine
    dependency chains.
  - Phase C unchanged structurally but fp16 (FWL hides the per-matmul
    weight switch that was exposed at f32r).
"""
import sys
sys.path.insert(0, "/opt/trn_rl_repo")
import numpy as np
import concourse.bass as bass
import concourse.mybir as mybir
import concourse.bacc as bacc
import concourse.tile as tile
from concourse.bass_utils import run_bass_kernel_spmd

B, S, D, H, HD = 4, 4096, 2048, 16, 128
NCORES = 8
F32, F16 = mybir.dt.float32, mybir.dt.float16
KT = D // 128             # contraction tiles (16)
SHIFT = 25.0              # constant softmax shift (softmax-invariant)
NEG = 1024.0              # additive mask magnitude for cross-token blocks
TA = 256                  # token chunk
GB = 4                    # 8-token blocks per phase-B group (32 tokens)
Exp = mybir.ActivationFunctionType.Exp
Axis = mybir.AxisListType
Alu = mybir.AluOpType


def mask_consts():
    # u8[r,(t,h)] = 1 if t==r ; v8[r,(t',g)] = -NEG*(1 - (t'==r))
    u = np.zeros((8, 128), np.float32)
    for r in range(8):
        u[r, r * 16:(r + 1) * 16] = 1.0
    v = np.full((8, 128), -NEG, np.float32)
    for r in range(8):
        v[r, r * 16:(r + 1) * 16] = 0.0
    return u, v


def build(T, repeat=1, trace_sim=False, phases="ABC"):
    TAe = min(TA, T)
    NCH = T // TAe            # chunks
    NG = TAe // (8 * GB)      # phase-B groups per chunk
    nc = bacc.Bacc(None, target_bir_lowering=False)
    dt_in = lambda n, s, dt=F16: nc.dram_tensor(n, s, dt, kind="ExternalInput")
    qT = dt_in("qT", [D, T]); kT = dt_in("kT", [D, T]); vT = dt_in("vT", [D, T])
    WqT = dt_in("WqT", [D, D]); WkT = dt_in("WkT", [D, D])
    WvT = dt_in("WvT", [D, D]); WoT = dt_in("WoT", [D, D])
    bqT = dt_in("bqT", [128, H], F32)
    bkT = dt_in("bkT", [128, H], F32)
    bvT = dt_in("bvT", [128, H], F32)
    bo_row = dt_in("bo_row", [1, D])
    ones_row = dt_in("ones_row", [1, 128])
    out_d = nc.dram_tensor("out", [T, D], F32, kind="ExternalOutput")

    u8_np, v8_np = mask_consts()
    u8_d = nc.inline_tensor(u8_np.astype(np.float16), "u8c")
    v8_d = nc.inline_tensor(v8_np.astype(np.float16), "v8c")
    id_d = nc.inline_tensor(np.eye(128, dtype=np.float16), "id128")

    with tile.TileContext(nc, trace_sim=trace_sim) as tc:
        with (
            tc.tile_pool(name="dram", bufs=1, space="DRAM") as dpool,
            tc.tile_pool(name="const", bufs=1) as cpool,
        ):
            QT_ds = [dpool.tile([128, TAe * H], F16, tag=f"QTd{i}", name=f"QTd{i}") for i in range(NCH)]
            KT_ds = [dpool.tile([128, TAe * H], F16, tag=f"KTd{i}", name=f"KTd{i}") for i in range(NCH)]
            VT_ds = [dpool.tile([128, TAe * H], F16, tag=f"VTd{i}", name=f"VTd{i}") for i in range(NCH)]
            ATT_ds = [dpool.tile([D, TAe], F16, tag=f"ATTd{i}", name=f"ATTd{i}") for i in range(NCH)]

            u8 = cpool.tile([8, 128], F16, tag="u8")
            v8 = cpool.tile([8, 128], F16, tag="v8")
            ident = cpool.tile([128, 128], F16, tag="ident")
            nc.sync.dma_start(u8[:], u8_d[:])
            nc.sync.dma_start(v8[:], v8_d[:])
            nc.sync.dma_start(ident[:], id_d[:])
            biasq = cpool.tile([128, H], F32, tag="bq")
            biask = cpool.tile([128, H], F32, tag="bk")
            biasv = cpool.tile([128, H], F32, tag="bvt")
            bor = cpool.tile([1, D], F16, tag="bo")
            onesr = cpool.tile([1, 128], F16, tag="ones")
            nc.sync.dma_start(biasq[:], bqT[:])
            nc.sync.dma_start(biask[:], bkT[:])
            nc.sync.dma_start(biasv[:], bvT[:])
            nc.sync.dma_start(bor[:], bo_row[:])
            nc.sync.dma_start(onesr[:], ones_row[:])
            shiftc = cpool.tile([128, 1], F32, tag="shiftc")
            nc.vector.memset(shiftc[:], -SHIFT)

            def _load_w(pool, win, tag):
                # quarter tiles -> first matmuls start after 1/4 load
                src = win.ap().rearrange("(it p) j -> p it j", p=128)
                parts = []
                for q in range(4):
                    wq = pool.tile([128, 4, D], F16, tag=f"{tag}{q}", name=f"{tag}{q}")
                    nc.sync.dma_start(wq[:], src[:, q * 4:(q + 1) * 4, :])
                    parts.append(wq)
                return parts

            def _phases():
                do_B = "B" in phases
                do_C = "C" in phases
                # ---------------- Phase A: projections ----------------
                with (
                    tc.tile_pool(name="wt", bufs=1) as wpool,
                    tc.tile_pool(name="xs", bufs=2) as xpool,
                    tc.tile_pool(name="psA", bufs=8, space="PSUM") as psA,
                    tc.tile_pool(name="stA", bufs=2) as stA,
                ):
                    for xin, win, bias, spills in (
                        (qT, WqT, biasq, QT_ds),
                        (kT, WkT, biask, KT_ds),
                        (vT, WvT, biasv, VT_ds),
                    ):
                        xs0 = xpool.tile([128, KT, TAe], F16, tag="xs", name="xs0")
                        nc.sync.dma_start(
                            xs0[:], xin[:, 0:TAe].rearrange("(it p) t -> p it t", p=128))
                        wt = _load_w(wpool, win, "wt")
                        for c in range(NCH):
                            if c == 0:
                                xs = xs0
                            else:
                                xs = xpool.tile([128, KT, TAe], F16, tag="xs")
                                nc.sync.dma_start(
                                    xs[:], xin[:, c * TAe:(c + 1) * TAe].rearrange(
                                        "(it p) t -> p it t", p=128))
                            stg = stA.tile([128, TAe, H], F16, tag="stA")
                            for jh in range(2):
                                pss = [psA.tile([128, TAe], F32, tag="psA",
                                                name=f"psA{jh}_{j}") for j in range(8)]
                                for q in range(4):
                                    for jl in range(8):
                                        jt = jh * 8 + jl
                                        for kl in range(4):
                                            ki = q * 4 + kl
                                            nc.tensor.matmul(
                                                pss[jl][:], wt[q][:, kl, jt * 128:(jt + 1) * 128],
                                                xs[:, ki, :], start=(ki == 0), stop=(ki == KT - 1))
                                for jl in range(8):
                                    jt = jh * 8 + jl
                                    nc.any.tensor_scalar_add(stg[:, :, jt], pss[jl][:],
                                                             bias[:, jt:jt + 1])
                            nc.sync.dma_start(
                                spills[c][:], stg[:].rearrange("p t h -> p (t h)"))

                # ---------------- Phase B (with Wo q0 prefetch) ----------------
                if not do_B:
                    return
                with tc.tile_pool(name="wo", bufs=1) as wopool:
                  wsrc = WoT.ap().rearrange("(h p) j -> p h j", p=128)
                  wo0 = wopool.tile([128, 4, D], F16, tag="wo0", name="wo0")
                  nc.sync.dma_start(wo0[:], wsrc[:, 0:4, :])
                  with (
                      tc.tile_pool(name="qk", bufs=2) as qkpool,
                      tc.tile_pool(name="vb", bufs=2) as vpool,
                      tc.tile_pool(name="attc", bufs=2) as apool,
                      tc.tile_pool(name="eb", bufs=3) as epool,
                      tc.tile_pool(name="wb", bufs=3) as wbpool,
                      tc.tile_pool(name="sm", bufs=4) as smpool,
                      tc.tile_pool(name="psS", bufs=2, space="PSUM") as psS,
                      tc.tile_pool(name="psT", bufs=2, space="PSUM") as psT,
                      tc.tile_pool(name="psV", bufs=2, space="PSUM") as psV,
                      tc.tile_pool(name="psA2", bufs=2, space="PSUM") as psA2,
                  ):
                      for c in range(NCH):
                          QTs = qkpool.tile([128, TAe, H], F16, tag="QTs")
                          KTs = qkpool.tile([128, TAe, H], F16, tag="KTs")
                          VTs = vpool.tile([128, TAe, H], F16, tag="VTs")
                          nc.sync.dma_start(
                              QTs[:], QT_ds[c][:].rearrange("p (t h) -> p t h", h=H))
                          nc.sync.dma_start(
                              KTs[:], KT_ds[c][:].rearrange("p (t h) -> p t h", h=H))
                          nc.sync.dma_start(
                              VTs[:], VT_ds[c][:].rearrange("p (t h) -> p t h", h=H))
                          ATTc = apool.tile([128, H, TAe], F16, tag="ATTc")
                          for g in range(NG):
                              t0 = g * 8 * GB

                              def blk(ts, b):
                                  sl = slice(t0 + b * 8, t0 + (b + 1) * 8)
                                  return ts[:, sl, :].rearrange("p t h -> p (t h)")

                              # V transposes (independent of score path)
                              ps_v = psV.tile([128, GB, 128], F16, tag="ps_v")
                              for b in range(GB):
                                  nc.tensor.matmul(ps_v[:, b, :], blk(VTs, b), ident[:],
                                                   is_transpose=True, skip_group_check=True)
                              Vb = vpool.tile([128, GB, 128], F16, tag="Vb")
                              nc.any.tensor_copy(
                                  Vb[:].rearrange("p b w -> p (b w)"),
                                  ps_v[:].rearrange("p b w -> p (b w)"))

                              # scores + mask (interleaved: start=True clears the
                              # whole bank's has_written bits, so each segment's
                              # score+mask pair must complete before the next start)
                              ps_s = psS.tile([128, GB, 128], F32, tag="ps_s")
                              for b in range(GB):
                                  nc.tensor.matmul(ps_s[:, b, :], blk(QTs, b), blk(KTs, b),
                                                   start=True, stop=False,
                                                   skip_group_check=True)
                                  nc.tensor.matmul(ps_s[:, b, :], u8[:], v8[:],
                                                   start=False, stop=True,
                                                   skip_group_check=True)

                              # exp (f32) + grouped softmax sums
                              E = epool.tile([128, GB, 128], F32, tag="E")
                              nc.scalar.activation(
                                  E[:].rearrange("p b w -> p (b w)"),
                                  ps_s[:].rearrange("p b w -> p (b w)"),
                                  Exp, bias=shiftc[:])
                              Z = smpool.tile([128, GB], F32, tag="Z")
                              nc.vector.tensor_reduce(Z[:], E[:], Axis.X, Alu.add)
                              R = smpool.tile([128, GB], F32, tag="R")
                              nc.vector.reciprocal(R[:], Z[:])
                              Wb = wbpool.tile([128, GB, 128], F16, tag="Wb")
                              for b in range(GB):
                                  nc.vector.tensor_scalar_mul(
                                      Wb[:, b, :], E[:, b, :], R[:, b:b + 1])

                              # transpose softmax weights
                              ps_t = psT.tile([128, GB, 128], F16, tag="ps_t")
                              for b in range(GB):
                                  nc.tensor.matmul(ps_t[:, b, :], Wb[:, b, :], ident[:],
                                                   is_transpose=True, skip_group_check=True)
                              WTs = wbpool.tile([128, GB, 128], F16, tag="WTs")
                              nc.any.tensor_copy(
                                  WTs[:].rearrange("p b w -> p (b w)"),
                                  ps_t[:].rearrange("p b w -> p (b w)"))

                              # attn = V^T @ W^T
                              ps_a = psA2.tile([128, GB, 128], F32, tag="ps_a")
                              for b in range(GB):
                                  nc.tensor.matmul(ps_a[:, b, :], Vb[:, b, :], WTs[:, b, :],
                                                   start=True, stop=True,
                                                   skip_group_check=True)
                              nc.any.tensor_copy(
                                  ATTc[:, :, t0:t0 + 8 * GB].rearrange(
                                      "p h (b t) -> p b t h", b=GB),
                                  ps_a[:].rearrange("p b (t h) -> p b t h", h=H))
                          nc.sync.dma_start(
                              ATT_ds[c][:].rearrange("(h p) t -> p h t", p=128), ATTc[:])

                  # ---------------- Phase C: output projection ----------------
                  if not do_C:
                      return
                  with (
                      tc.tile_pool(name="ca", bufs=2) as capool,
                      tc.tile_pool(name="psC", bufs=8, space="PSUM") as psC,
                      tc.tile_pool(name="stC", bufs=4) as stC,
                  ):
                      ATTs0 = capool.tile([128, H, TAe], F16, tag="ATTs", name="ATTs0")
                      nc.sync.dma_start(
                          ATTs0[:], ATT_ds[0][:].rearrange("(h p) t -> p h t", p=128))
                      wo = [wo0]
                      for q in range(1, 4):
                          wq = wopool.tile([128, 4, D], F16, tag=f"wo{q}", name=f"wo{q}")
                          nc.sync.dma_start(wq[:], wsrc[:, q * 4:(q + 1) * 4, :])
                          wo.append(wq)
                      for cc in range(NCH):
                          if cc == 0:
                              ATTs = ATTs0
                          else:
                              ATTs = capool.tile([128, H, TAe], F16, tag="ATTs")
                              nc.sync.dma_start(
                                  ATTs[:], ATT_ds[cc][:].rearrange("(h p) t -> p h t", p=128))
                          for tt in range(TAe // 128):
                              pss = [psC.tile([128, 512], F32, tag="psC",
                                              name=f"psC{tt}_{j}") for j in range(4)]
                              for hq in range(4):
                                  for jc in range(4):
                                      for hl in range(4):
                                          h = hq * 4 + hl
                                          nc.tensor.matmul(
                                              pss[jc][:], ATTs[:, h, tt * 128:(tt + 1) * 128],
                                              wo[hq][:, hl, jc * 512:(jc + 1) * 512],
                                              start=(h == 0), stop=False)
                              for jc in range(4):
                                  nc.tensor.matmul(pss[jc][:], onesr[:],
                                                   bor[:, jc * 512:(jc + 1) * 512],
                                                   start=False, stop=True)
                                  st = stC.tile([128, 512], F32, tag="stC")
                                  nc.any.tensor_copy(st[:], pss[jc][:])
                                  nc.sync.dma_start(
                                      out_d[cc * TAe + tt * 128: cc * TAe + (tt + 1) * 128,
                                            jc * 512:(jc + 1) * 512], st[:])

            for _rep in range(repeat):
                _phases()
    nc.compile()
    return nc


_cache = {}


def get_nc(T):
    if T not in _cache:
        _cache[T] = build(T)
    return _cache[T]


def make_in_maps(q, k, v, Wq, bq, Wk, bk, Wv, bv, Wo, bo, ncores=NCORES, T=None):
    f, h = np.float32, np.float16
    q = np.asarray(q, f).reshape(-1, D)
    k = np.asarray(k, f).reshape(-1, D)
    v = np.asarray(v, f).reshape(-1, D)
    if T is None:
        T = q.shape[0] // ncores
    WqT = np.ascontiguousarray(np.asarray(Wq, f).T).astype(h)
    WkT = np.ascontiguousarray(np.asarray(Wk, f).T).astype(h)
    WvT = np.ascontiguousarray(np.asarray(Wv, f).T).astype(h)
    WoT = np.ascontiguousarray(np.asarray(Wo, f).T).astype(h)
    bqT = np.ascontiguousarray(np.asarray(bq, f).reshape(H, 128).T)
    bkT = np.ascontiguousarray(np.asarray(bk, f).reshape(H, 128).T)
    bvTc = np.ascontiguousarray(np.asarray(bv, f).reshape(H, 128).T)
    bor = np.asarray(bo, f).reshape(1, D).astype(h)
    maps = []
    for c in range(ncores):
        sl = slice(c * T, (c + 1) * T)
        maps.append({
            "qT": np.ascontiguousarray(q[sl].T).astype(h),
            "kT": np.ascontiguousarray(k[sl].T).astype(h),
            "vT": np.ascontiguousarray(v[sl].T).astype(h),
            "WqT": WqT, "WkT": WkT, "WvT": WvT, "WoT": WoT,
            "bqT": bqT, "bkT": bkT, "bvT": bvTc, "bo_row": bor,
            "ones_row": np.ones((1, 128), h),
        })
    return maps, T


_runner = None


def _get_runner(nc):
    """Compile the 8-core sharded executable once; reuse across kernel() calls
    (run_bass_kernel_spmd re-traces and re-compiles on every invocation)."""
    global _runner
    if _runner is not None:
        return _runner
    import jax
    from jax.sharding import Mesh, PartitionSpec, NamedSharding
    from jax.experimental.shard_map import shard_map
    from concourse.bass2jax import (
        install_neuronx_cc_hook, partition_id_tensor, _bass_exec_p)

    install_neuronx_cc_hook()
    partition_name = nc.partition_id_tensor.name if nc.partition_id_tensor else None
    in_names, out_names, out_avals, zero_shapes = [], [], [], []
    for alloc in nc.m.functions[0].allocations:
        if not isinstance(alloc, mybir.MemoryLocationSet):
            continue
        name = alloc.memorylocations[0].name
        if alloc.kind == "ExternalInput":
            if name != partition_name:
                in_names.append(name)
        elif alloc.kind == "ExternalOutput":
            out_names.append(name)
            shape = tuple(alloc.tensor_shape)
            dtype = mybir.dt.np(alloc.dtype)
            out_avals.append(jax.core.ShapedArray(shape, dtype))
            zero_shapes.append((shape, dtype))
    n_params, n_outs = len(in_names), len(out_avals)
    all_names = list(in_names) + list(out_names)
    if partition_name is not None:
        all_names.append(partition_name)
    donate = tuple(range(n_params, n_params + n_outs))

    def _body(*args):
        operands = list(args)
        if partition_name is not None:
            operands.append(partition_id_tensor())
        return tuple(_bass_exec_p.bind(
            *operands, out_avals=tuple(out_avals), in_names=tuple(all_names),
            out_names=tuple(out_names), lowering_input_output_aliases=(),
            sim_require_finite=True, sim_require_nnan=True, nc=nc))

    devices = jax.devices()[:NCORES]
    mesh = Mesh(np.asarray(devices), ("core",))
    sharded = jax.jit(
        shard_map(_body, mesh=mesh,
                  in_specs=(PartitionSpec("core"),) * (n_params + n_outs),
                  out_specs=(PartitionSpec("core"),) * n_outs,
                  check_rep=False),
        donate_argnums=donate, keep_unused=True)
    shd = NamedSharding(mesh, PartitionSpec("core"))

    def run(maps):
        import jax
        concat_in = [
            np.concatenate([np.asarray(m[name]) for m in maps], axis=0)
            for name in in_names
        ]
        dev_in = [jax.device_put(a, shd) for a in concat_in]
        zeros = [
            jax.device_put(np.zeros((NCORES * s[0], *s[1:]), d), shd)
            for (s, d) in zero_shapes
        ]
        out_arrs = sharded(*dev_in, *zeros)
        return [
            {name: np.asarray(out_arrs[i]).reshape(NCORES, *out_avals[i].shape)[c]
             for i, name in enumerate(out_names)}
            for c in range(NCORES)
        ]

    _runner = run
    return run


def kernel(q, k, v, Wq, bq, Wk, bk, Wv, bv, Wo, bo):
    maps, T = make_in_maps(q, k, v, Wq, bq, Wk, bk, Wv, bv, Wo, bo)
    nc = get_nc(T)
    results = _get_runner(nc)(maps)
    out = np.concatenate([np.asarray(r["out"]) for r in results], axis=0)
    return out.reshape(B, S, D).astype(np.float32)



# revision 48
# speedup vs baseline: 6.1963x; 6.1963x over previous
"""Trainium2 Bass kernel v4 for nn_MultiHeadAttention_67379446939752.

Per-token multi-head attention, data-parallel over tokens across 8 cores.

Structure (all tile pools live in one flat scope so the Tile dataflow
scheduler can overlap everything):

  A   q,k projections feature-partitioned; v projection token-partitioned
      (so phase B consumes V as the attn stationary with no PE transposes).
      Strips of 8 j-tiles packed two-per-PSUM-bank using the pending-zero
      overwrite (start=True once per bank).  Hybrid strip order: ki-outer on
      the first chunk after a weight switch (matches the weight eighths
      streaming in just-in-time), j-outer afterwards (spreads the PSUM->SBUF
      copies so the next strip never waits on a copy burst).
  B1  scores -> softmax -> transposed weights, needs only Q,K: emitted
      between k and v so its work fills the k->v weight-switch stall; the
      transposed weights spill to DRAM so B1 can run arbitrarily far ahead.
  B2  attn = V^T @ W^T, needs V; follows the v chunks.
  C   output projection, 4 full-bank [128,512] strips per half-chunk,
      Wo streamed through the same weight slots (first two eighths
      double-buffered so they prefetch during v).

  Weights stream as eighths [128, 2, 2048] fp16; tags we0/we1 live in a
  bufs=2 pool so the next matrix's leading k-tiles prefetch during the
  previous one.  PSUM: 4 accumulation banks shared by A and C + 2 score
  banks + 1 transpose bank + 1 attn bank = 8.

  Mask matmul of v2 replaced by tensor_tensor_reduce with a 0/1
  block-diagonal mask (masked exp sum on DVE); all hot PSUM->SBUF copies
  pinned to DVE; exp on ACT.  Per-column biases (v, o) only emitted when
  biases are nonzero (they are zero for this problem).
"""
import sys
sys.path.insert(0, "/opt/trn_rl_repo")
import numpy as np
import concourse.bass as bass
import concourse.mybir as mybir
import concourse.bacc as bacc
import concourse.tile as tile

B, S, D, H, HD = 4, 4096, 2048, 16, 128
NCORES = 8
F32, F16 = mybir.dt.float32, mybir.dt.float16
KT = D // 128              # contraction tiles (16)
SHIFT = 25.0               # constant softmax shift (softmax-invariant)
TA = 256                   # phase-A token chunk
HC = 128                   # phase-B/C token half-chunk
GB = 4                     # 8-token blocks per phase-B group (32 tokens)
Exp = mybir.ActivationFunctionType.Exp
Axis = mybir.AxisListType
Alu = mybir.AluOpType


def mask_const():
    # M[(t,h),(t',g)] = 1 if t==t' else 0 for 8-token blocks of 16 heads
    m = np.zeros((128, 128), np.float32)
    for t in range(8):
        m[t * 16:(t + 1) * 16, t * 16:(t + 1) * 16] = 1.0
    return m


def build(T, repeat=1, trace_sim=False, phases="ABC", use_bias=False):
    NCH = T // TA             # phase-A chunks (8)
    NHC = T // HC             # phase-B/C half-chunks (16)
    NG = HC // (8 * GB)       # phase-B groups per half-chunk (4)
    nc = bacc.Bacc(None, target_bir_lowering=False)
    dt_in = lambda n, s, dt=F16: nc.dram_tensor(n, s, dt, kind="ExternalInput")
    qT = dt_in("qT", [D, T]); kT = dt_in("kT", [D, T]); vT = dt_in("vT", [D, T])
    WqT = dt_in("WqT", [D, D]); WkT = dt_in("WkT", [D, D])
    WvT = dt_in("WvT", [D, D]); WoT = dt_in("WoT", [D, D])
    bqT = dt_in("bqT", [128, H], F32)
    bkT = dt_in("bkT", [128, H], F32)
    bv_row = dt_in("bv_row", [1, D])
    bo_row = dt_in("bo_row", [1, D])
    ones_row = dt_in("ones_row", [1, 128])
    out_d = nc.dram_tensor("out", [T, D], F32, kind="ExternalOutput")

    m_d = nc.inline_tensor(mask_const(), "mblk")
    id_d = nc.inline_tensor(np.eye(128, dtype=np.float16), "id128")

    from contextlib import ExitStack
    with tile.TileContext(nc, trace_sim=trace_sim) as tc:
        with ExitStack() as stack:
            ep = stack.enter_context
            dpool = ep(tc.tile_pool(name="dram", bufs=1, space="DRAM"))
            cpool = ep(tc.tile_pool(name="const", bufs=1))
            wpool2 = ep(tc.tile_pool(name="w2", bufs=2))   # eighths 0,1 (dbl-buf)
            wpool1 = ep(tc.tile_pool(name="w1", bufs=1))   # eighths 2..7
            xpool = ep(tc.tile_pool(name="xs", bufs=2))
            stpool = ep(tc.tile_pool(name="stg", bufs=2))
            accp = ep(tc.tile_pool(name="acc", bufs=1, space="PSUM"))   # 4 banks
            psS = ep(tc.tile_pool(name="psS", bufs=3, space="PSUM"))    # 3 banks
            psTp = ep(tc.tile_pool(name="psT", bufs=1, space="PSUM"))   # 1 bank
            qkpool = ep(tc.tile_pool(name="qk", bufs=2))
            vpool = ep(tc.tile_pool(name="vtp", bufs=2))
            epool = ep(tc.tile_pool(name="eb", bufs=3))
            empool = ep(tc.tile_pool(name="em", bufs=2))
            wbpool = ep(tc.tile_pool(name="wb", bufs=2))
            wtpool = ep(tc.tile_pool(name="wt", bufs=2))
            smpool = ep(tc.tile_pool(name="sm", bufs=2))
            apool = ep(tc.tile_pool(name="attc", bufs=3))
            scpool = ep(tc.tile_pool(name="stc", bufs=4))
            QT_ds = [dpool.tile([128, TA * H], F16, tag=f"QTd{i}", name=f"QTd{i}")
                     for i in range(NCH)]
            KT_ds = [dpool.tile([128, TA * H], F16, tag=f"KTd{i}", name=f"KTd{i}")
                     for i in range(NCH)]
            V_d = dpool.tile([T, D], F16, tag="Vd", name="Vd")

            mblk4 = cpool.tile([128, GB, 128], F32, tag="mblk4")
            ident = cpool.tile([128, 128], F16, tag="ident")
            for b in range(GB):
                nc.gpsimd.dma_start(mblk4[:, b, :], m_d[:])
            nc.gpsimd.dma_start(ident[:], id_d[:])
            biasq = cpool.tile([128, H], F32, tag="bq")
            biask = cpool.tile([128, H], F32, tag="bk")
            nc.gpsimd.dma_start(biasq[:], bqT[:])
            nc.gpsimd.dma_start(biask[:], bkT[:])
            shiftc = cpool.tile([128, 1], F32, tag="shiftc")
            nc.vector.memset(shiftc[:], -SHIFT)
            if use_bias:
                bvr = cpool.tile([1, D], F16, tag="bvr")
                bor = cpool.tile([1, D], F16, tag="bor")
                onesr = cpool.tile([1, 128], F16, tag="ones")
                nc.gpsimd.dma_start(bvr[:], bv_row[:])
                nc.gpsimd.dma_start(bor[:], bo_row[:])
                nc.gpsimd.dma_start(onesr[:], ones_row[:])

            def load_w(win, eng0=None, eng1=None):
                """Load a [D, D] weight as 8 eighth tiles [128, 2, D].

                kt0 halves issue on eng0 (default SP), kt1 on eng1 (default
                ACT): two DMA queues stream the weight in parallel, matching
                the ki-outer just-in-time consumption rate."""
                eng0 = eng0 or nc.sync
                eng1 = eng1 or nc.scalar
                parts = []
                for e in range(8):
                    pool = wpool2 if e < 2 else wpool1
                    wp = pool.tile([128, 2, D], F16, tag=f"we{e}",
                                   name=f"we{e}")
                    src = win.ap()[e * 256:(e + 1) * 256, :].rearrange(
                        "(kt p) j -> p kt j", p=128)
                    eng0.dma_start(wp[:, 0, :], src[:, 0, :])
                    eng1.dma_start(wp[:, 1, :], src[:, 1, :])
                    parts.append(wp)
                return parts

            def wsl(parts, ki, j0, j1):
                return parts[ki // 2][:, ki % 2, j0:j1]

            def _phases():
                do_B = "B" in phases
                do_C = "C" in phases

                # ---------- phase A helpers ----------
                # xs tiles hold a 512-token chunk PAIR so every projection
                # matmul has a 512-wide moving operand (the ~28ns/MM NX
                # dispatch floor is real on HW; half the instructions).
                PAIR = 2 * TA
                NPR = T // PAIR

                def load_xs(xin, pr):
                    xs = xpool.tile([128, KT, PAIR], F16, tag="xs", name="xs")
                    src = xin[:, pr * PAIR:(pr + 1) * PAIR].rearrange(
                        "(it p) t -> p it t", p=128)
                    for qq in range(8):
                        eng = nc.sync if qq % 2 == 0 else nc.scalar
                        eng.dma_start(xs[:, qq * 2:(qq + 1) * 2, :],
                                      src[:, qq * 2:(qq + 1) * 2, :])
                    return xs

                def proj_qk(xin, bias, spills, first_xs, wparts):
                    for pr in range(NPR):
                        xs = first_xs if pr == 0 else load_xs(xin, pr)
                        stgt = [stpool.tile([128, TA, H], F16, tag="stg",
                                            name=f"stgt{i}") for i in range(2)]
                        for sg in range(4):
                            bank = [accp.tile([128, PAIR], F32, tag=f"acc{bk}",
                                              name=f"a{sg}_{bk}")
                                    for bk in range(4)]

                            def mm(ki, j):
                                jt = sg * 4 + j
                                nc.tensor.matmul(
                                    bank[j][:],
                                    wsl(wparts, ki, jt * 128, (jt + 1) * 128),
                                    xs[:, ki, :],
                                    start=(ki == 0), stop=(ki == KT - 1),
                                    skip_group_check=True)

                            if pr == 0 and sg == 0:  # weight-JIT order
                                for ki in range(KT):
                                    for j in range(4):
                                        mm(ki, j)
                            else:        # copy-spreading order
                                for j in range(4):
                                    for ki in range(KT):
                                        mm(ki, j)
                            for j in range(4):
                                jt = sg * 4 + j
                                for i in range(2):
                                    nc.vector.tensor_scalar_add(
                                        stgt[i][:, :, jt],
                                        bank[j][:, i * TA:(i + 1) * TA],
                                        bias[:, jt:jt + 1])
                        for i in range(2):
                            nc.gpsimd.dma_start(
                                spills[pr * 2 + i][:],
                                stgt[i][:].rearrange("p t h -> p (t h)"))

                def proj_v(first_xs, wparts):
                    for pr in range(NPR):
                        xs = first_xs if pr == 0 else load_xs(vT, pr)
                        for tb in range(PAIR // 128):
                            stgv = stpool.tile([128, 8, 256], F16, tag="stg",
                                               name="stgv")
                            bank = [accp.tile([128, 512], F32, tag=f"acc{bk}",
                                              name=f"v{tb}_{bk}")
                                    for bk in range(4)]
                            st_ap = lambda ki: xs[:, ki,
                                                  tb * 128:(tb + 1) * 128]

                            def mmv(ki, j):
                                nc.tensor.matmul(
                                    bank[j][:],
                                    st_ap(ki),
                                    wsl(wparts, ki, j * 512, (j + 1) * 512),
                                    start=(ki == 0),
                                    stop=(ki == KT - 1 and not use_bias),
                                    skip_group_check=True)

                            if pr == 0 and tb == 0:
                                for ki in range(KT):
                                    for j in range(4):
                                        mmv(ki, j)
                            else:
                                for j in range(4):
                                    for ki in range(KT):
                                        mmv(ki, j)
                            if use_bias:
                                for j in range(4):
                                    nc.tensor.matmul(
                                        bank[j][:],
                                        onesr[:],
                                        bvr[:, j * 512:(j + 1) * 512],
                                        start=False, stop=True,
                                        skip_group_check=True)
                            for j in range(4):
                                nc.vector.tensor_copy(
                                    stgv[:, 2 * j:2 * j + 2, :].rearrange(
                                        "p a f -> p (a f)"),
                                    bank[j][:])
                            r0 = pr * PAIR + tb * 128
                            nc.gpsimd.dma_start(
                                V_d[r0:r0 + 128, :],
                                stgv[:].rearrange("p j f -> p (j f)"))

                # ---------- phase B + C, software-pipelined ----------
                # Per half-chunk hc, the B softmax chain (scores -> exp ->
                # masked sum -> 1/Z -> weights -> transpose -> attn) is
                # emitted one C-strip at a time between the previous
                # half-chunk's four C j-strips: every cross-engine hop's
                # latency hides under a 3.4us block of C matmuls.  B stages
                # inside one hc are skewed (scores g, transpose g-1,
                # attn g-2) for the same reason.
                def bc(do_C):
                    state = {}

                    def b_loads(hc):
                        c, half = divmod(hc, 2)
                        QTs = qkpool.tile([128, HC, H], F16, tag="QTs",
                                          name="QTs")
                        KTs = qkpool.tile([128, HC, H], F16, tag="KTs",
                                          name="KTs")
                        nc.sync.dma_start(
                            QTs[:],
                            QT_ds[c][:, half * HC * H:(half + 1) * HC * H]
                            .rearrange("p (t h) -> p t h", h=H))
                        nc.scalar.dma_start(
                            KTs[:],
                            KT_ds[c][:, half * HC * H:(half + 1) * HC * H]
                            .rearrange("p (t h) -> p t h", h=H))
                        Vtp = vpool.tile([128, HC // 8, 128], F16, tag="Vtp",
                                         name="Vtp")
                        nc.sync.dma_start(
                            Vtp[:],
                            V_d[hc * HC:(hc + 1) * HC, :].rearrange(
                                "(b ti) (g hd) -> (ti g) b hd", ti=8, g=H))
                        ATTc = apool.tile([128, H, HC], F16, tag="ATTc",
                                          name="ATTc")
                        state[hc] = dict(QTs=QTs, KTs=KTs, Vtp=Vtp, ATTc=ATTc,
                                         Wb={}, WTs={})

                    def blk(ts, bi):
                        return ts[:, bi * 8:(bi + 1) * 8, :].rearrange(
                            "p t h -> p (t h)")

                    def b_scores(hc, g):
                        st = state[hc]
                        ps_s = psS.tile([128, GB, 128], F32, tag="s",
                                        name=f"s{hc}_{g}")
                        for b in range(GB):
                            nc.tensor.matmul(
                                ps_s[:, b, :], blk(st["QTs"], g * GB + b),
                                blk(st["KTs"], g * GB + b),
                                start=(b == 0), stop=(b == GB - 1),
                                skip_group_check=True)
                        E = epool.tile([128, GB, 128], F32, tag="E", name="E")
                        nc.scalar.activation(
                            E[:].rearrange("p b w -> p (b w)"),
                            ps_s[:].rearrange("p b w -> p (b w)"),
                            Exp, bias=shiftc[:])
                        Em = empool.tile([128, GB, 128], F32, tag="Em",
                                         name="Em")
                        ZR = smpool.tile([128, 2, GB], F32, tag="ZR", name="ZR")
                        # Em = E * blockdiag-mask; Z = row-sums per block
                        nc.vector.tensor_mul(
                            Em[:].rearrange("p b w -> p (b w)"),
                            E[:].rearrange("p b w -> p (b w)"),
                            mblk4[:].rearrange("p b w -> p (b w)"))
                        nc.vector.tensor_reduce(
                            ZR[:, 0, :], Em[:], Axis.X, Alu.add)
                        nc.vector.reciprocal(ZR[:, 1, :], ZR[:, 0, :])
                        Wb = wbpool.tile([128, GB, 128], F16, tag="Wb",
                                         name="Wb")
                        for b in range(GB):
                            nc.vector.tensor_scalar_mul(
                                Wb[:, b, :], Em[:, b, :], ZR[:, 1, b:b + 1])
                        st["Wb"][g] = Wb

                    def b_transpose(hc, g):
                        st = state[hc]
                        ps_t = psTp.tile([128, GB, 128], F16, tag="t",
                                         name=f"t{hc}_{g}")
                        for b in range(GB):
                            nc.tensor.matmul(ps_t[:, b, :], st["Wb"][g][:, b, :],
                                             ident[:], is_transpose=True,
                                             skip_group_check=True)
                        WTs = wtpool.tile([128, GB, 128], F16, tag="WTs",
                                          name="WTs")
                        nc.vector.tensor_copy(
                            WTs[:].rearrange("p b w -> p (b w)"),
                            ps_t[:].rearrange("p b w -> p (b w)"))
                        st["WTs"][g] = WTs

                    def b_attn(hc, g):
                        st = state[hc]
                        ps_a = accp.tile([128, GB, 128], F32,
                                         tag=f"acc{2 + g % 2}",
                                         name=f"pa{hc}_{g}")
                        for b in range(GB):
                            nc.tensor.matmul(
                                ps_a[:, b, :], st["Vtp"][:, g * GB + b, :],
                                st["WTs"][g][:, b, :],
                                start=(b == 0), stop=(b == GB - 1),
                                skip_group_check=True)
                        nc.vector.tensor_copy(
                            st["ATTc"][:, :, g * 8 * GB:(g + 1) * 8 * GB]
                            .rearrange("p h (b t) -> p b t h", b=GB),
                            ps_a[:].rearrange("p b (t h) -> p b t h", h=H))

                    def c_strip(hc, j):
                        ATTc = state[hc]["ATTc"]
                        bank = accp.tile([128, 512], F32, tag=f"acc{j % 2}",
                                         name=f"c{hc}_{j}")
                        for h in range(KT):
                            nc.tensor.matmul(
                                bank[:], ATTc[:, h, :],
                                wsl(wo, h, j * 512, (j + 1) * 512),
                                start=(h == 0),
                                stop=(h == KT - 1 and not use_bias),
                                skip_group_check=True)
                        if use_bias:
                            nc.tensor.matmul(
                                bank[:], onesr[:],
                                bor[:, j * 512:(j + 1) * 512],
                                start=False, stop=True,
                                skip_group_check=True)
                        st = scpool.tile([128, 512], F32, tag="stC", name="stC")
                        nc.vector.tensor_copy(st[:], bank[:])
                        nc.gpsimd.dma_start(
                            out_d[hc * HC:(hc + 1) * HC,
                                  j * 512:(j + 1) * 512], st[:])

                    def c_done(hc):
                        del state[hc]

                    # first two hcs' loads go ahead of Wo's (slot-blocked)
                    # weight DMAs in the SP/ACT queues
                    b_loads(0)
                    b_loads(1)
                    # Wo streams via the Pool queue, idle during the A->BC
                    # transition (v spills done, out writes not yet started)
                    wo = load_w(WoT, nc.gpsimd, nc.gpsimd) if do_C else None
                    for hc in range(NHC):
                        if hc >= 1 and hc + 1 < NHC:
                            b_loads(hc + 1)
                        for g in range(NG):
                            b_scores(hc, g)
                            if g >= 1:
                                b_transpose(hc, g - 1)
                            if g >= 2:
                                b_attn(hc, g - 2)
                            if do_C and hc >= 2:
                                c_strip(hc - 2, g)
                        b_transpose(hc, NG - 1)
                        b_attn(hc, NG - 2)
                        b_attn(hc, NG - 1)
                        if do_C and hc >= 2:
                            c_done(hc - 2)
                    if do_C:
                        for hc in (NHC - 2, NHC - 1):
                            for j in range(NG):
                                c_strip(hc, j)
                            c_done(hc)

                # ---------- emission order ----------
                # Cold start: interleave the first xs parts with the weight
                # eighths on both queues so the first matmul fires ~4us in
                # and ki consumption tracks the two-queue arrival rate.
                xs0 = xpool.tile([128, KT, PAIR], F16, tag="xs", name="xs0")
                srcx = qT[:, 0:PAIR].rearrange("(it p) t -> p it t", p=128)
                wq = []
                wsrc = []
                for e in range(8):
                    pool = wpool2 if e < 2 else wpool1
                    wp = pool.tile([128, 2, D], F16, tag=f"we{e}", name=f"we{e}")
                    wq.append(wp)
                    wsrc.append(WqT.ap()[e * 256:(e + 1) * 256, :].rearrange(
                        "(kt p) j -> p kt j", p=128))
                nc.sync.dma_start(wq[0][:, 0, :], wsrc[0][:, 0, :])
                nc.scalar.dma_start(xs0[:, 2:4, :], srcx[:, 2:4, :])
                nc.sync.dma_start(xs0[:, 0:2, :], srcx[:, 0:2, :])
                nc.scalar.dma_start(wq[0][:, 1, :], wsrc[0][:, 1, :])
                for e in range(1, 8):
                    nc.sync.dma_start(wq[e][:, 0, :], wsrc[e][:, 0, :])
                    if e < 4:
                        nc.sync.dma_start(
                            xs0[:, 2 + 2 * e:4 + 2 * e, :],
                            srcx[:, 2 + 2 * e:4 + 2 * e, :])
                        nc.scalar.dma_start(
                            xs0[:, 8 + 2 * e:10 + 2 * e, :],
                            srcx[:, 8 + 2 * e:10 + 2 * e, :])
                    nc.scalar.dma_start(wq[e][:, 1, :], wsrc[e][:, 1, :])
                proj_qk(qT, biasq, QT_ds, xs0, wq)
                xs0 = load_xs(kT, 0)
                proj_qk(kT, biask, KT_ds, xs0, load_w(WkT))
                xs0 = load_xs(vT, 0)
                proj_v(xs0, load_w(WvT))
                if do_B:
                    bc(do_C)

            for _rep in range(repeat):
                _phases()
    nc.compile()
    return nc


_cache = {}


def get_nc(T, use_bias=False):
    key = (T, use_bias)
    if key not in _cache:
        _cache[key] = build(T, use_bias=use_bias)
    return _cache[key]


def make_in_maps(q, k, v, Wq, bq, Wk, bk, Wv, bv, Wo, bo, ncores=NCORES, T=None):
    f, h = np.float32, np.float16
    q = np.asarray(q, f).reshape(-1, D)
    k = np.asarray(k, f).reshape(-1, D)
    v = np.asarray(v, f).reshape(-1, D)
    if T is None:
        T = q.shape[0] // ncores
    WqT = np.ascontiguousarray(np.asarray(Wq, f).T).astype(h)
    WkT = np.ascontiguousarray(np.asarray(Wk, f).T).astype(h)
    WvT = np.ascontiguousarray(np.asarray(Wv, f).T).astype(h)
    WoT = np.ascontiguousarray(np.asarray(Wo, f).T).astype(h)
    bqT = np.ascontiguousarray(np.asarray(bq, f).reshape(H, 128).T)
    bkT = np.ascontiguousarray(np.asarray(bk, f).reshape(H, 128).T)
    bvr = np.asarray(bv, f).reshape(1, D).astype(h)
    bor = np.asarray(bo, f).reshape(1, D).astype(h)
    maps = []
    for c in range(ncores):
        sl = slice(c * T, (c + 1) * T)
        maps.append({
            "qT": np.ascontiguousarray(q[sl].T).astype(h),
            "kT": np.ascontiguousarray(k[sl].T).astype(h),
            "vT": np.ascontiguousarray(v[sl].T).astype(h),
            "WqT": WqT, "WkT": WkT, "WvT": WvT, "WoT": WoT,
            "bqT": bqT, "bkT": bkT, "bv_row": bvr, "bo_row": bor,
            "ones_row": np.ones((1, 128), h),
        })
    return maps, T


def _use_bias(bq, bk, bv, bo):
    return any(np.any(np.asarray(b)) for b in (bq, bk, bv, bo))


_runner = None


def _get_runner(nc):
    """Compile the 8-core sharded executable once; reuse across kernel() calls
    (run_bass_kernel_spmd re-traces and re-compiles on every invocation)."""
    global _runner
    if _runner is not None:
        return _runner
    import jax
    from jax.sharding import Mesh, PartitionSpec, NamedSharding
    from jax.experimental.shard_map import shard_map
    from concourse.bass2jax import (
        install_neuronx_cc_hook, partition_id_tensor, _bass_exec_p)

    install_neuronx_cc_hook()
    partition_name = nc.partition_id_tensor.name if nc.partition_id_tensor else None
    in_names, out_names, out_avals, zero_shapes = [], [], [], []
    for alloc in nc.m.functions[0].allocations:
        if not isinstance(alloc, mybir.MemoryLocationSet):
            continue
        name = alloc.memorylocations[0].name
        if alloc.kind == "ExternalInput":
            if name != partition_name:
                in_names.append(name)
        elif alloc.kind == "ExternalOutput":
            out_names.append(name)
            shape = tuple(alloc.tensor_shape)
            dtype = mybir.dt.np(alloc.dtype)
            out_avals.append(jax.core.ShapedArray(shape, dtype))
            zero_shapes.append((shape, dtype))
    n_params, n_outs = len(in_names), len(out_avals)
    all_names = list(in_names) + list(out_names)
    if partition_name is not None:
        all_names.append(partition_name)
    donate = tuple(range(n_params, n_params + n_outs))

    def _body(*args):
        operands = list(args)
        if partition_name is not None:
            operands.append(partition_id_tensor())
        return tuple(_bass_exec_p.bind(
            *operands, out_avals=tuple(out_avals), in_names=tuple(all_names),
            out_names=tuple(out_names), lowering_input_output_aliases=(),
            sim_require_finite=True, sim_require_nnan=True, nc=nc))

    devices = jax.devices()[:NCORES]
    mesh = Mesh(np.asarray(devices), ("core",))
    sharded = jax.jit(
        shard_map(_body, mesh=mesh,
                  in_specs=(PartitionSpec("core"),) * (n_params + n_outs),
                  out_specs=(PartitionSpec("core"),) * n_outs,
                  check_rep=False),
        donate_argnums=donate, keep_unused=True)
    shd = NamedSharding(mesh, PartitionSpec("core"))

    def run(maps):
        import jax
        concat_in = [
            np.concatenate([np.asarray(m[name]) for m in maps], axis=0)
            for name in in_names
        ]
        dev_in = [jax.device_put(a, shd) for a in concat_in]
        zeros = [
            jax.device_put(np.zeros((NCORES * s[0], *s[1:]), d), shd)
            for (s, d) in zero_shapes
        ]
        out_arrs = sharded(*dev_in, *zeros)
        return [
            {name: np.asarray(out_arrs[i]).reshape(NCORES, *out_avals[i].shape)[c]
             for i, name in enumerate(out_names)}
            for c in range(NCORES)
        ]

    _runner = run
    return run


def kernel(q, k, v, Wq, bq, Wk, bk, Wv, bv, Wo, bo):
    maps, T = make_in_maps(q, k, v, Wq, bq, Wk, bk, Wv, bv, Wo, bo)
    nc = get_nc(T, use_bias=_use_bias(bq, bk, bv, bo))
    results = _get_runner(nc)(maps)
    out = np.concatenate([np.asarray(r["out"]) for r in results], axis=0)
    return out.reshape(B, S, D).astype(np.float32)
